# revision 1
# baseline (speedup 1.0000x reference)
"""Trainium2 Bass kernel for the cross-attention transformer block.

Strategy (8 NeuronCores, data-parallel over batch B=8, one batch item per core):
  - Feature-major activations ([feature, token]); host pre-transposes, bf16
    inputs only (no fp32 copies of x/y).
  - LayerNorm: stats via ones-matmuls on PE (1/C folded into the ones vector,
    all bf16), rstd chain on broadcast tiles, 2-pass in-place bf16 apply.
  - Attention per (head-group g of 4 heads, 512-token block nb): scores via
    row-tiled matmuls (K=32 each, one PSUM bank per concurrent matmul),
    FD=1024 exp on ACT, AV with augmented V (ones column -> softmax
    denominator) packed 2 heads per PSUM bank via partition-offset col
    tiling.
  - Phase interleaving by emission priority: QKV projections for later head
    groups fill PE gaps during attention of the first 512 tokens; the FFN of
    the first 512 tokens fills PE gaps during attention of the second 512
    tokens.  This keeps TensorE continuously busy so the HAM clock gate stays
    at 8/8 (2.4 GHz) instead of half-clocking through the attention phase.
  - FFN W3 runs ct-outer with small [128,128] weight tiles so it needs only
    1 PSUM bank, leaving room for the attention/FFN overlap window.
"""

import sys

for _p in ("/opt/trn_rl_repo", "/root/.axon_site/_ro/trn_rl_repo"):
    if _p not in sys.path:
        sys.path.append(_p)

import numpy as np
import ml_dtypes

import concourse.bacc as bacc
import concourse.mybir as mybir
from concourse.tile import TileContext
from concourse import bass_utils

F32 = mybir.dt.float32
BF16 = mybir.dt.bfloat16
AF = mybir.ActivationFunctionType
OP = mybir.AluOpType

P = 128
B, N, C, H, D, W = 8, 1024, 1024, 16, 32, 4
HD = H * D            # 512
DA = 2 * D            # 64: per-head stride in vaug (V cols 0..31, ones col 32)
FF = W * C            # 4096
KT = C // P           # 8 feature k-tiles
NBS = 512             # token block (attention + FFN)
EPS = 1e-5
NCORES = 8

_BUILD_CACHE = {}
_LAST_IN_MAPS = None


def _build(flags):
    f_g1, f_g2, f_g3, f_b1, f_b2, f_b3 = flags
    nc = bacc.Bacc("TRN2", target_bir_lowering=False)

    xTb = nc.dram_tensor("xTb", [C, N], BF16, kind="ExternalInput")
    yTb = nc.dram_tensor("yTb", [C, N], BF16, kind="ExternalInput")
    wq = nc.dram_tensor("wq", [C, HD], BF16, kind="ExternalInput")
    wk = nc.dram_tensor("wk", [C, HD], BF16, kind="ExternalInput")
    wv = nc.dram_tensor("wv", [C, HD], BF16, kind="ExternalInput")
    w1 = nc.dram_tensor("w1", [HD, C], BF16, kind="ExternalInput")
    w2 = nc.dram_tensor("w2", [C, FF], BF16, kind="ExternalInput")
    w3 = nc.dram_tensor("w3", [FF, C], BF16, kind="ExternalInput")
    vecs = {}
    if f_g1:
        vecs["g1"] = nc.dram_tensor("g1", [C, 1], F32, kind="ExternalInput")
        vecs["be1"] = nc.dram_tensor("be1", [C, 1], F32, kind="ExternalInput")
    if f_g2:
        vecs["g2"] = nc.dram_tensor("g2", [C, 1], F32, kind="ExternalInput")
        vecs["be2"] = nc.dram_tensor("be2", [C, 1], F32, kind="ExternalInput")
    if f_g3:
        vecs["g3"] = nc.dram_tensor("g3", [C, 1], F32, kind="ExternalInput")
        vecs["be3"] = nc.dram_tensor("be3", [C, 1], F32, kind="ExternalInput")
    if f_b1:
        vecs["b1"] = nc.dram_tensor("b1", [C, 1], F32, kind="ExternalInput")
    if f_b2:
        vecs["b2"] = nc.dram_tensor("b2", [FF, 1], F32, kind="ExternalInput")
    if f_b3:
        vecs["b3"] = nc.dram_tensor("b3", [C, 1], F32, kind="ExternalInput")
    OT = nc.dram_tensor("OT", [C, N], F32, kind="ExternalOutput")

    with TileContext(nc) as tc:
        with (
            tc.tile_pool(name="p_main", bufs=1) as p_main,
            tc.tile_pool(name="ps_sc", bufs=2, space="PSUM") as ps_sc,
            tc.tile_pool(name="ps_aug", bufs=2, space="PSUM") as ps_aug,
            tc.tile_pool(name="ps_w", bufs=2, space="PSUM") as ps_w,
        ):
            # ---------------- constants ----------------
            ones_mean = p_main.tile([P, 1], BF16, name="ones_mean", tag="cst",
                                    bufs=4)
            nc.vector.memset(ones_mean, 1.0 / C)
            ones_row = p_main.tile([1, P], BF16, name="ones_row", tag="cst",
                                   bufs=4)
            nc.vector.memset(ones_row, 1.0)
            eps_tile = p_main.tile([P, 1], F32, name="eps_tile", tag="cst",
                                   bufs=4)
            nc.vector.memset(eps_tile, EPS)
            # e4s[32j, 32j:32j+32] = 1: broadcast Z row at partition 32j of
            # zcat to output partitions 32j..32j+31
            e4s = p_main.tile([P, P], BF16, name="e4s", tag="cst", bufs=4)
            nc.vector.memset(e4s, 0.0)
            for j in range(4):
                nc.vector.memset(e4s[32 * j:32 * j + 1, 32 * j:32 * (j + 1)],
                                 1.0)

            vec_tiles = {}
            for vn, dram in vecs.items():
                nparts = dram.shape[0] // P
                tiles = []
                for k in range(nparts):
                    t = p_main.tile([P, 1], F32, name=f"{vn}_{k}", tag="vec",
                                    bufs=nparts + 8)
                    nc.sync.dma_start(out=t, in_=dram[k * P:(k + 1) * P, 0:1])
                    tiles.append(t)
                vec_tiles[vn] = tiles

            # ---------------- helpers ----------------
            def emit_ln(src, width, gb, name, st_pool, st_tag):
                """In-place feature-major layernorm of bf16 tiles `src`."""
                nblk = width // 512
                kt = len(src)
                srows = [p_main.tile([1, width], BF16, name=f"{name}_srow{i}",
                                     tag="rows", bufs=2) for i in range(2)]
                # s1 = mean(x), per 512-token block
                for bb in range(nblk):
                    sp = st_pool.tile([1, 512], F32, name=f"{name}_sp0{bb}",
                                      tag=st_tag)
                    for k in range(kt):
                        nc.tensor.matmul(
                            sp, ones_mean[:, 0:1],
                            src[k][:, bb * 512:(bb + 1) * 512],
                            start=(k == 0), stop=(k == kt - 1))
                    nc.vector.tensor_copy(
                        out=srows[0][0:1, bb * 512:(bb + 1) * 512], in_=sp)
                # s2 = mean(x^2): stream small square tiles
                for bb in range(nblk):
                    sp = st_pool.tile([1, 512], F32, name=f"{name}_sp1{bb}",
                                      tag=st_tag)
                    for k in range(kt):
                        sqt = p_main.tile([P, 512], BF16,
                                          name=f"{name}_sq{bb}{k}", tag="sq",
                                          bufs=2)
                        nc.vector.tensor_mul(
                            out=sqt, in0=src[k][:, bb * 512:(bb + 1) * 512],
                            in1=src[k][:, bb * 512:(bb + 1) * 512])
                        nc.tensor.matmul(
                            sp, ones_mean[:, 0:1], sqt,
                            start=(k == 0), stop=(k == kt - 1))
                    nc.vector.tensor_copy(
                        out=srows[1][0:1, bb * 512:(bb + 1) * 512], in_=sp)
                bstats = []
                for idx in range(2):
                    dst = p_main.tile([P, width], BF16, name=f"{name}_b{idx}",
                                      tag="stb", bufs=3)
                    for bb in range(nblk):
                        bp = st_pool.tile([P, 512], F32,
                                          name=f"{name}_bp{idx}{bb}",
                                          tag=st_tag)
                        nc.tensor.matmul(
                            bp, ones_row[0:1, :],
                            srows[idx][0:1, bb * 512:(bb + 1) * 512],
                            start=True, stop=True)
                        nc.vector.tensor_copy(
                            out=dst[:, bb * 512:(bb + 1) * 512], in_=bp)
                    bstats.append(dst)
                mu_b, ex2_b = bstats
                rstd = p_main.tile([P, width], BF16, name=f"{name}_rst",
                                   tag="stb", bufs=3)
                for bb in range(nblk):
                    sl = slice(bb * 512, (bb + 1) * 512)
                    var = p_main.tile([P, 512], F32, name=f"{name}_var{bb}",
                                      tag="stf", bufs=2)
                    nc.vector.tensor_mul(out=var, in0=mu_b[:, sl],
                                         in1=mu_b[:, sl])
                    nc.vector.tensor_tensor(out=var, in0=ex2_b[:, sl],
                                            in1=var, op=OP.subtract)
                    nc.scalar.activation(out=var, in_=var, func=AF.Sqrt,
                                         bias=eps_tile[:, 0:1])
                    rstd_f = p_main.tile([P, 512], F32, name=f"{name}_rsf{bb}",
                                         tag="stf", bufs=2)
                    nc.vector.reciprocal_approx_fast(out=rstd_f, in_=var)
                    nc.vector.tensor_copy(out=rstd[:, sl], in_=rstd_f)
                for k in range(kt):
                    nc.vector.tensor_tensor(out=src[k], in0=src[k], in1=mu_b,
                                            op=OP.subtract)
                    nc.vector.tensor_tensor(out=src[k], in0=src[k], in1=rstd,
                                            op=OP.mult)
                    if gb is not None:
                        nc.vector.tensor_scalar(
                            out=src[k], in0=src[k], scalar1=gb[0][k],
                            scalar2=gb[1][k], op0=OP.mult, op1=OP.add)

            # ---------------- persistent activation tiles ----------------
            xnb = []
            for k in range(KT):
                t = p_main.tile([P, N], BF16, name=f"x{k}", tag="xn", bufs=KT)
                eng = nc.scalar if k % 2 == 0 else nc.gpsimd
                eng.dma_start(out=t, in_=xTb[k * P:(k + 1) * P, :])
                xnb.append(t)
            qT = [p_main.tile([P, N], BF16, name=f"qT{g}", tag="qT", bufs=4)
                  for g in range(4)]
            kTt = [p_main.tile([P, N], BF16, name=f"kT{g}", tag="kT", bufs=4)
                   for g in range(4)]
            oT = [p_main.tile([P, N], BF16, name=f"oT{g}", tag="oT", bufs=4)
                  for g in range(4)]
            vaug = [p_main.tile([P, H * DA], BF16, name=f"v{mt}", tag="vg",
                                bufs=KT) for mt in range(KT)]
            out1 = [p_main.tile([P, N], BF16, name=f"out1_{k}", tag="out1",
                                bufs=KT) for k in range(KT)]
            w1_sb = []
            for g in range(4):
                t = p_main.tile([P, C], BF16, name=f"w1s{g}", tag="w1", bufs=4)
                nc.sync.dma_start(out=t, in_=w1[g * P:(g + 1) * P, :])
                w1_sb.append(t)

            def emit_qk_proj(g, w_sb, src, dst, pfx):
                for bb in range(2):
                    pp = ps_w.tile([P, 512], F32, name=f"pp_{pfx}{g}{bb}",
                                   tag="psw")
                    for k in range(KT):
                        nc.tensor.matmul(
                            pp, w_sb[k][:, g * P:(g + 1) * P],
                            src[k][:, bb * 512:(bb + 1) * 512],
                            start=(k == 0), stop=(k == KT - 1))
                    nc.vector.tensor_copy(
                        out=dst[g][:, bb * 512:(bb + 1) * 512], in_=pp)

            def emit_attn(g, nb):
                ns = slice(nb * NBS, (nb + 1) * NBS)
                # two head-pair accumulators: pair pp holds heads 2pp (parts
                # 0:33) and 2pp+1 (parts 64:97); col-tiled concurrent AV
                augp = [ps_aug.tile([P, NBS], F32, name=f"aug{g}{nb}{pp}",
                                    tag="aug") for pp in range(2)]
                for mt in range(KT):
                    e_sb = []
                    for pr in range(2):
                        sp = ps_sc.tile([P, 1024], F32,
                                        name=f"s{g}{nb}{mt}{pr}", tag="sc")
                        for jj in range(2):
                            j = 2 * pr + jj
                            nc.tensor.matmul(
                                sp[:, jj * NBS:(jj + 1) * NBS],
                                kTt[g][32 * j:32 * (j + 1),
                                       mt * P:(mt + 1) * P],
                                qT[g][32 * j:32 * (j + 1), ns],
                                start=True, stop=True,
                                tile_position=(32 * j, 0))
                        et = p_main.tile([P, 1024], BF16,
                                         name=f"e{g}{nb}{mt}{pr}", tag="e",
                                         bufs=2)
                        nc.scalar.activation(out=et, in_=sp, func=AF.Exp)
                        e_sb.append(et)
                    for j in range(4):
                        hh = 4 * g + j
                        nc.tensor.matmul(
                            augp[j // 2][64 * (j % 2):64 * (j % 2) + 33, :],
                            vaug[mt][:, hh * DA:hh * DA + 33],
                            e_sb[j // 2][:, (j % 2) * NBS:(j % 2 + 1) * NBS],
                            start=(mt == 0), stop=(mt == KT - 1),
                            skip_group_check=True)
                # drain: assemble o_un + Z rows, normalize
                zcat = p_main.tile([P, NBS], BF16, name=f"zc{g}{nb}", tag="zc",
                                   bufs=2)
                nc.vector.memset(zcat, 0.0)
                for j in range(4):
                    nc.vector.tensor_copy(
                        out=zcat[32 * j:32 * j + 1, :],
                        in_=augp[j // 2][64 * (j % 2) + 32:64 * (j % 2) + 33,
                                         :])
                o_un = p_main.tile([P, NBS], BF16, name=f"ou{g}{nb}", tag="zc",
                                   bufs=2)
                for j in range(4):
                    nc.vector.tensor_copy(
                        out=o_un[32 * j:32 * (j + 1), :],
                        in_=augp[j // 2][64 * (j % 2):64 * (j % 2) + 32, :])
                zb = ps_w.tile([P, NBS], F32, name=f"zb{g}{nb}", tag="psw")
                nc.tensor.matmul(zb, e4s, zcat, start=True, stop=True)
                zsb = p_main.tile([P, NBS], F32, name=f"zs{g}{nb}", tag="zi",
                                  bufs=2)
                nc.vector.tensor_copy(out=zsb, in_=zb)
                zinv = p_main.tile([P, NBS], F32, name=f"zi{g}{nb}", tag="zi",
                                   bufs=2)
                nc.vector.reciprocal_approx_fast(out=zinv, in_=zsb)
                nc.vector.tensor_tensor(out=oT[g][:, ns], in0=o_un, in1=zinv,
                                        op=OP.mult)

            def emit_w1_ln3(nb):
                ns = slice(nb * NBS, (nb + 1) * NBS)
                o1b = []
                for ct in range(KT):
                    up = ps_w.tile([P, NBS], F32, name=f"u{nb}{ct}", tag="psw")
                    for g in range(4):
                        nc.tensor.matmul(
                            up, w1_sb[g][:, ct * P:(ct + 1) * P],
                            oT[g][:, ns], start=(g == 0), stop=(g == 3))
                    b1s = vec_tiles["b1"][ct] if f_b1 else 0.0
                    nc.vector.scalar_tensor_tensor(
                        out=out1[ct][:, ns], in0=up, scalar=b1s,
                        in1=xnb[ct][:, ns], op0=OP.add, op1=OP.add)
                    ob = p_main.tile([P, NBS], BF16, name=f"o1b{nb}{ct}",
                                     tag="o1b", bufs=8)
                    nc.vector.tensor_copy(out=ob, in_=out1[ct][:, ns])
                    o1b.append(ob)
                gb3 = (vec_tiles["g3"], vec_tiles["be3"]) if f_g3 else None
                emit_ln(o1b, NBS, gb3, f"ln3{nb}", ps_w, "psw")
                return o1b

            def emit_w2_chunk(nb, fq, ln3, h1g):
                w2t = []
                for ct in range(KT):
                    t = p_main.tile([P, NBS], BF16, name=f"w2_{nb}{fq}{ct}",
                                    tag="w2", bufs=12)
                    nc.sync.dma_start(
                        out=t,
                        in_=w2[ct * P:(ct + 1) * P, fq * 512:(fq + 1) * 512])
                    w2t.append(t)
                for fi in range(4):
                    ft = fq * 4 + fi
                    hp = ps_sc.tile([P, NBS], F32, name=f"h{nb}{ft}", tag="sc")
                    for ct in range(KT):
                        nc.tensor.matmul(
                            hp, w2t[ct][:, fi * P:(fi + 1) * P], ln3[ct],
                            start=(ct == 0), stop=(ct == KT - 1))
                    hg = p_main.tile([P, NBS], BF16, name=f"h1_{nb}{ft}",
                                     tag="h1", bufs=32)
                    b2s = vec_tiles["b2"][ft] if f_b2 else 0.0
                    nc.scalar.activation(out=hg, in_=hp, func=AF.Gelu,
                                         bias=b2s)
                    h1g.append(hg)

            w3r = w3.rearrange("(f p) c -> p f c", p=P)

            def emit_w3_ct_range(nb, h1g, ct0, ct1):
                ns = slice(nb * NBS, (nb + 1) * NBS)
                for ct in range(ct0, ct1):
                    h2 = ps_w.tile([P, NBS], F32, name=f"h2_{nb}{ct}",
                                   tag="psw")
                    for q in range(4):
                        w3t = p_main.tile([P, 8 * P], BF16,
                                          name=f"w3_{nb}{ct}{q}", tag="w3s",
                                          bufs=3)
                        w3t3 = w3t.rearrange("p (f c) -> p f c", c=P)
                        nc.gpsimd.dma_start(
                            out=w3t3,
                            in_=w3r[:, 8 * q:8 * (q + 1),
                                    ct * P:(ct + 1) * P])
                        for fl in range(8):
                            ft = 8 * q + fl
                            nc.tensor.matmul(
                                h2, w3t[:, fl * P:(fl + 1) * P], h1g[ft],
                                start=(ft == 0), stop=(ft == FF // P - 1))
                    fin = p_main.tile([P, NBS], F32, name=f"fin{nb}{ct}",
                                      tag="fin", bufs=1)
                    b3s = vec_tiles["b3"][ct] if f_b3 else 0.0
                    nc.vector.scalar_tensor_tensor(
                        out=fin, in0=h2, scalar=b3s,
                        in1=out1[ct][:, ns], op0=OP.add, op1=OP.add)
                    nc.gpsimd.dma_start(out=OT[ct * P:(ct + 1) * P, ns],
                                        in_=fin)

            # ---------------- phase A: inputs, LN, projections --------------
            with tc.tile_pool(name="p_a", bufs=1) as p_a:
                ynb = []
                for k in range(KT):
                    t = p_a.tile([P, N], BF16, name=f"y{k}", tag="yn", bufs=KT)
                    eng = nc.gpsimd if k % 2 == 0 else nc.scalar
                    eng.dma_start(out=t, in_=yTb[k * P:(k + 1) * P, :])
                    ynb.append(t)
                wq_sb, wk_sb, wv_sb = [], [], []
                for wn, dram, lst in (("wk", wk, wk_sb), ("wv", wv, wv_sb),
                                      ("wq", wq, wq_sb)):
                    for k in range(KT):
                        t = p_a.tile([P, HD], BF16, name=f"{wn}s{k}", tag=wn,
                                     bufs=KT)
                        nc.sync.dma_start(out=t,
                                          in_=dram[k * P:(k + 1) * P, :])
                        lst.append(t)

                gb2 = (vec_tiles["g2"], vec_tiles["be2"]) if f_g2 else None
                emit_ln(ynb, N, gb2, "lny", ps_sc, "sc")
                gb1 = (vec_tiles["g1"], vec_tiles["be1"]) if f_g1 else None
                emit_ln(xnb, N, gb1, "lnx", ps_sc, "sc")

                emit_qk_proj(0, wk_sb, ynb, kTt, 'k')
                emit_qk_proj(0, wq_sb, xnb, qT, 'q')
                # V: token-major with ones columns (softmax denominator)
                for mt in range(KT):
                    vp = ps_w.tile([P, HD], F32, name=f"vp{mt}", tag="psw")
                    for k in range(KT):
                        nc.tensor.matmul(
                            vp, ynb[k][:, mt * P:(mt + 1) * P], wv_sb[k],
                            start=(k == 0), stop=(k == KT - 1))
                    nc.vector.memset(vaug[mt], 1.0)
                    v3 = vaug[mt].rearrange("p (h w) -> p h w", w=DA)
                    nc.vector.tensor_copy(
                        out=v3[:, :, 0:D],
                        in_=vp.rearrange("p (h w) -> p h w", w=D))

                for g in range(4):
                    emit_attn(g, 0)
                    if g < 3:
                        emit_qk_proj(g + 1, wk_sb, ynb, kTt, 'k')
                        emit_qk_proj(g + 1, wq_sb, xnb, qT, 'q')

            # ---------------- tail: W2 alone; attn(nb1) overlaps W3(nb0) ----
            ln3_0 = emit_w1_ln3(0)
            h1g_0 = []
            for fq in range(8):
                emit_w2_chunk(0, fq, ln3_0, h1g_0)
            for g in range(4):
                emit_attn(g, 1)
                emit_w3_ct_range(0, h1g_0, 2 * g, 2 * g + 2)
            ln3_1 = emit_w1_ln3(1)
            h1g_1 = []
            for fq in range(8):
                emit_w2_chunk(1, fq, ln3_1, h1g_1)
            emit_w3_ct_range(1, h1g_1, 0, KT)

    nc.finalize()
    return nc


def _nontrivial(v, val):
    return not np.allclose(np.asarray(v), val, rtol=0.0, atol=0.0)


def kernel(x, y, Wq, Wk, Wv, W1, b1, g1, be1, g2, be2, g3, be3, W2, b2, W3, b3):
    x = np.asarray(x, np.float32)
    y = np.asarray(y, np.float32)
    bf = ml_dtypes.bfloat16

    f_g1 = _nontrivial(g1, 1.0) or _nontrivial(be1, 0.0)
    f_g2 = _nontrivial(g2, 1.0) or _nontrivial(be2, 0.0)
    f_g3 = _nontrivial(g3, 1.0) or _nontrivial(be3, 0.0)
    f_b1 = _nontrivial(b1, 0.0)
    f_b2 = _nontrivial(b2, 0.0)
    f_b3 = _nontrivial(b3, 0.0)
    flags = (f_g1, f_g2, f_g3, f_b1, f_b2, f_b3)

    if flags not in _BUILD_CACHE:
        _BUILD_CACHE[flags] = _build(flags)
    nc = _BUILD_CACHE[flags]

    # stacked per-head weights [H, C, D] -> [C, H*D]; attention scale folded
    # into Wq
    wq_h = (np.transpose(np.asarray(Wq, np.float32), (1, 0, 2))
            .reshape(C, HD) * (D ** -0.5)).astype(bf)
    wk_h = np.transpose(np.asarray(Wk, np.float32), (1, 0, 2)).reshape(C, HD).astype(bf)
    wv_h = np.transpose(np.asarray(Wv, np.float32), (1, 0, 2)).reshape(C, HD).astype(bf)
    w1_h = np.asarray(W1, np.float32).astype(bf)
    w2_h = np.asarray(W2, np.float32).astype(bf)
    w3_h = np.asarray(W3, np.float32).astype(bf)

    in_maps = []
    for b in range(NCORES):
        m = {
            "xTb": np.ascontiguousarray(x[b].T).astype(bf),
            "yTb": np.ascontiguousarray(y[b].T).astype(bf),
            "wq": wq_h, "wk": wk_h, "wv": wv_h,
            "w1": w1_h, "w2": w2_h, "w3": w3_h,
        }
        if f_g1:
            m["g1"] = np.asarray(g1, np.float32).reshape(C, 1)
            m["be1"] = np.asarray(be1, np.float32).reshape(C, 1)
        if f_g2:
            m["g2"] = np.asarray(g2, np.float32).reshape(C, 1)
            m["be2"] = np.asarray(be2, np.float32).reshape(C, 1)
        if f_g3:
            m["g3"] = np.asarray(g3, np.float32).reshape(C, 1)
            m["be3"] = np.asarray(be3, np.float32).reshape(C, 1)
        if f_b1:
            m["b1"] = np.asarray(b1, np.float32).reshape(C, 1)
        if f_b2:
            m["b2"] = np.asarray(b2, np.float32).reshape(FF, 1)
        if f_b3:
            m["b3"] = np.asarray(b3, np.float32).reshape(C, 1)
        in_maps.append(m)

    global _LAST_IN_MAPS
    _LAST_IN_MAPS = in_maps
    res = bass_utils.run_bass_kernel_spmd(nc, in_maps, core_ids=list(range(NCORES)))
    out = np.stack([np.ascontiguousarray(r["OT"].T) for r in res.results])
    return out.astype(np.float32)

